# revision 6
# baseline (speedup 1.0000x reference)
"""GAT (3-head, edge-weighted) message-passing kernel for 8 Trainium2 NeuronCores.

Strategy: sort edges by destination on host, give each core a contiguous
128-aligned destination-node range (49 windows x 128 nodes). Each core:
  phase 1: XT[n] = [x@W_lin+b | s_src(3) | s_dst(3)] (+ compact SDS[n,4] table)
  phase 2: per 128-edge tile: indirect-gather XT[src] and SDS[dst], softmax
           numerator p = exp(leakyrelu(s_src+s_dst)), one-hot scatter matmuls
           accumulate per-window denom [128,3] and agg [128f, 3*128n] in PSUM,
           per-window: project agg_h @ W_h scaled by 1/denom, write out rows.
No collectives needed: every core owns its dst range end-to-end.
"""

import numpy as np
import concourse.bass as bass
import concourse.bacc as bacc
import concourse.mybir as mybir
from concourse.tile import TileContext
from concourse import bass_utils

F32 = mybir.dt.float32
I32 = mybir.dt.int32

N_NODES = 50000
N_EDGES = 600000
DIM = 128
N_HEADS = 3
NEG_SLOPE = 0.2
NCORES = 8
NPW = 128                      # nodes per window
WPC = 49                       # windows per core
NPC = NPW * WPC                # 6272 nodes per core
NPAD = NPC * NCORES            # 50176
PADIDX = NPAD                  # poison row index (s_src = -1e4 -> p = 0)
NROWS = NPAD + 128             # 50304 rows in XT/SDS, 393 tiles of 128
NT1 = NROWS // 128             # phase-1 tiles

_cache = {}


def _phase1(nc, tc, cpool, xp, xt, sds, wg_sb, biasr_sb, poi_sb):
    with (
        tc.tile_pool(name="p1", bufs=6) as p1,
        tc.tile_pool(name="p1ps", bufs=4, space="PSUM") as p1ps,
    ):
        for i in range(NT1):
            xpt = p1.tile([128, DIM], F32, tag="xpt")
            nc.sync.dma_start(out=xpt[:], in_=xp[i * 128:(i + 1) * 128, :])
            ps = p1ps.tile([128, 138], F32, tag="ps1")
            nc.tensor.matmul(out=ps[:], lhsT=xpt[:], rhs=wg_sb[:],
                             start=True, stop=True)
            row = p1.tile([128, 138], F32, tag="row")
            nc.vector.tensor_tensor(out=row[:], in0=ps[:], in1=biasr_sb[:],
                                    op=mybir.AluOpType.add)
            nc.sync.dma_start(out=xt[i * 128:(i + 1) * 128, :],
                              in_=row[:, 0:134])
            nc.sync.dma_start(out=sds[i * 128:(i + 1) * 128, :],
                              in_=row[:, 134:138])
        # poison row: padding edges get s_src=-1e4 -> p = 0
        nc.sync.dma_start(out=xt[PADIDX:PADIDX + 1, 128:131], in_=poi_sb[:])


def _phase2(nc, tc, K, xt, sds, srca, dstg, dstc, ewa, ddia, outc,
            iota_sb, wh_sb, bias3_sb):
    with (
        tc.tile_pool(name="win", bufs=3) as wpool,
        tc.tile_pool(name="edge", bufs=8) as epool,
        tc.tile_pool(name="mst", bufs=4) as mpool,
        tc.tile_pool(name="oh", bufs=6) as ohpool,
        tc.tile_pool(name="sm", bufs=8) as smpool,
        tc.tile_pool(name="fl", bufs=3) as flpool,
        tc.tile_pool(name="aggps", bufs=2, space="PSUM") as aggps,
        tc.tile_pool(name="denps", bufs=2, space="PSUM") as denps,
        tc.tile_pool(name="ops", bufs=3, space="PSUM") as ops,
    ):
        for w in range(WPC):
            r0 = w * 128
            srcw = wpool.tile([128, K], I32, tag="srcw")
            nc.sync.dma_start(out=srcw[:], in_=srca[r0:r0 + 128, :])
            dstgw = wpool.tile([128, K], I32, tag="dstgw")
            nc.sync.dma_start(out=dstgw[:], in_=dstg[r0:r0 + 128, :])
            dstcw = wpool.tile([128, K], F32, tag="dstcw")
            nc.sync.dma_start(out=dstcw[:], in_=dstc[r0:r0 + 128, :])
            eww = wpool.tile([128, K], F32, tag="eww")
            nc.sync.dma_start(out=eww[:], in_=ewa[r0:r0 + 128, :])
            ddiw = wpool.tile([128, K], F32, tag="ddiw")
            nc.sync.dma_start(out=ddiw[:], in_=ddia[r0:r0 + 128, :])
            ewn = wpool.tile([128, K], F32, tag="ewn")
            nc.vector.tensor_tensor(out=ewn[:], in0=eww[:], in1=ddiw[:],
                                    op=mybir.AluOpType.subtract)

            ps_den = denps.tile([128, 3], F32, tag="den")
            ps_agg = aggps.tile([128, N_HEADS * 128], F32, tag="agg")

            for t in range(K):
                xtg = epool.tile([128, 134], F32, tag="xtg")
                nc.gpsimd.indirect_dma_start(
                    out=xtg[:], out_offset=None, in_=xt[:],
                    in_offset=bass.IndirectOffsetOnAxis(
                        ap=srcw[:, t:t + 1], axis=0))
                sdg = epool.tile([128, 4], F32, tag="sdg")
                nc.gpsimd.indirect_dma_start(
                    out=sdg[:], out_offset=None, in_=sds[:],
                    in_offset=bass.IndirectOffsetOnAxis(
                        ap=dstgw[:, t:t + 1], axis=0))

                onehot = ohpool.tile([128, 128], F32, tag="onehot")
                nc.vector.tensor_scalar(
                    out=onehot[:], in0=iota_sb[:],
                    scalar1=dstcw[:, t:t + 1], scalar2=None,
                    op0=mybir.AluOpType.is_equal)

                e1 = smpool.tile([128, 3], F32, tag="e1")
                nc.vector.tensor_tensor(out=e1[:], in0=xtg[:, 128:131],
                                        in1=sdg[:, 0:3],
                                        op=mybir.AluOpType.add)
                sc = smpool.tile([128, 3], F32, tag="sc")
                nc.vector.tensor_scalar(
                    out=sc[:], in0=e1[:], scalar1=NEG_SLOPE, scalar2=None,
                    op0=mybir.AluOpType.mult)
                t2 = smpool.tile([128, 3], F32, tag="t2")
                nc.vector.tensor_tensor(out=t2[:], in0=e1[:], in1=sc[:],
                                        op=mybir.AluOpType.max)
                p = smpool.tile([128, 3], F32, tag="p")
                nc.scalar.activation(out=p[:], in_=t2[:],
                                     func=mybir.ActivationFunctionType.Exp)

                nc.tensor.matmul(out=ps_den[:], lhsT=onehot[:], rhs=p[:],
                                 start=(t == 0), stop=(t == K - 1))

                q = smpool.tile([128, 3], F32, tag="q")
                nc.vector.tensor_scalar(
                    out=q[:], in0=p[:], scalar1=ewn[:, t:t + 1],
                    scalar2=None, op0=mybir.AluOpType.mult)

                mst = mpool.tile([128, N_HEADS * 128], F32, tag="mst")
                for h in range(N_HEADS):
                    nc.vector.tensor_scalar(
                        out=mst[:, h * 128:(h + 1) * 128], in0=iota_sb[:],
                        scalar1=dstcw[:, t:t + 1],
                        scalar2=q[:, h:h + 1],
                        op0=mybir.AluOpType.is_equal,
                        op1=mybir.AluOpType.mult)

                nc.tensor.matmul(out=ps_agg[:], lhsT=xtg[:, 0:128],
                                 rhs=mst[:], start=(t == 0),
                                 stop=(t == K - 1))

            # window flush
            den = flpool.tile([128, 3], F32, tag="dens")
            nc.vector.tensor_scalar(
                out=den[:], in0=ps_den[:], scalar1=1e-16, scalar2=3.0,
                op0=mybir.AluOpType.max, op1=mybir.AluOpType.mult)
            inv = flpool.tile([128, 3], F32, tag="inv")
            nc.vector.reciprocal(out=inv[:], in_=den[:])
            agg = flpool.tile([128, N_HEADS * 128], F32, tag="aggs")
            nc.vector.tensor_copy(out=agg[:], in_=ps_agg[:])

            acc = flpool.tile([128, DIM], F32, tag="acc")
            tmp = flpool.tile([128, DIM], F32, tag="tmp")
            for h in range(N_HEADS):
                ps_o = ops.tile([128, DIM], F32, tag="ps_o")
                nc.tensor.matmul(out=ps_o[:],
                                 lhsT=agg[:, h * 128:(h + 1) * 128],
                                 rhs=wh_sb[:, h * DIM:(h + 1) * DIM],
                                 start=True, stop=True)
                dst_t = acc if h == 0 else tmp
                nc.vector.tensor_scalar(
                    out=dst_t[:], in0=ps_o[:], scalar1=inv[:, h:h + 1],
                    scalar2=None, op0=mybir.AluOpType.mult)
                if h > 0:
                    nc.vector.tensor_tensor(out=acc[:], in0=acc[:],
                                            in1=tmp[:],
                                            op=mybir.AluOpType.add)
            out_sb = flpool.tile([128, DIM], F32, tag="outsb")
            nc.vector.tensor_tensor(out=out_sb[:], in0=acc[:],
                                    in1=bias3_sb[:],
                                    op=mybir.AluOpType.add)
            nc.sync.dma_start(out=outc[r0:r0 + 128, :], in_=out_sb[:])


def _build(K):
    nc = bacc.Bacc("TRN2", target_bir_lowering=False, debug=False,
                   num_devices=NCORES)

    xp = nc.dram_tensor("xp", [NT1 * 128, DIM], F32, kind="ExternalInput")
    wg = nc.dram_tensor("wg", [DIM, 138], F32, kind="ExternalInput")
    biasr = nc.dram_tensor("biasr", [128, 138], F32, kind="ExternalInput")
    wheads = nc.dram_tensor("wheads", [N_HEADS * DIM, DIM], F32,
                            kind="ExternalInput")
    bias3 = nc.dram_tensor("bias3", [128, DIM], F32, kind="ExternalInput")
    iota = nc.dram_tensor("iota", [128, 128], F32, kind="ExternalInput")
    poison = nc.dram_tensor("poison", [1, 3], F32, kind="ExternalInput")
    srca = nc.dram_tensor("srca", [WPC * 128, K], I32, kind="ExternalInput")
    dstg = nc.dram_tensor("dstg", [WPC * 128, K], I32, kind="ExternalInput")
    dstc = nc.dram_tensor("dstc", [WPC * 128, K], F32, kind="ExternalInput")
    ewa = nc.dram_tensor("ewa", [WPC * 128, K], F32, kind="ExternalInput")
    ddia = nc.dram_tensor("ddia", [WPC * 128, K], F32, kind="ExternalInput")

    xt = nc.dram_tensor("xt", [NROWS, 134], F32)
    sds = nc.dram_tensor("sds", [NROWS, 4], F32)
    outc = nc.dram_tensor("outc", [NPC, DIM], F32, kind="ExternalOutput")

    with TileContext(nc) as tc:
        with tc.tile_pool(name="const", bufs=1) as cpool:
            wg_sb = cpool.tile([DIM, 138], F32, tag="wg")
            nc.sync.dma_start(out=wg_sb[:], in_=wg[:])
            biasr_sb = cpool.tile([128, 138], F32, tag="biasr")
            nc.sync.dma_start(out=biasr_sb[:], in_=biasr[:])
            iota_sb = cpool.tile([128, 128], F32, tag="iota")
            nc.sync.dma_start(out=iota_sb[:], in_=iota[:])
            wh_sb = cpool.tile([128, N_HEADS * DIM], F32, tag="wh")
            for h in range(N_HEADS):
                nc.sync.dma_start(out=wh_sb[:, h * DIM:(h + 1) * DIM],
                                  in_=wheads[h * DIM:(h + 1) * DIM, :])
            bias3_sb = cpool.tile([128, DIM], F32, tag="bias3")
            nc.sync.dma_start(out=bias3_sb[:], in_=bias3[:])
            poi_sb = cpool.tile([1, 3], F32, tag="poi")
            nc.sync.dma_start(out=poi_sb[:], in_=poison[:])

            _phase1(nc, tc, cpool, xp, xt, sds, wg_sb, biasr_sb, poi_sb)
            _phase2(nc, tc, K, xt, sds, srca, dstg, dstc, ewa, ddia, outc,
                    iota_sb, wh_sb, bias3_sb)

    nc.compile()
    return nc


def _prep(x, edge_index, edge_ids, ddi_weight, W_lin, b_lin, edge_emb,
          W_heads, att_src, att_dst, bias_heads):
    x = np.asarray(x, np.float32)
    src = np.asarray(edge_index[0], np.int64)
    dst = np.asarray(edge_index[1], np.int64)
    eids = np.asarray(edge_ids, np.int64)
    ddi = np.asarray(ddi_weight, np.float32)
    W_lin = np.asarray(W_lin, np.float32)
    b_lin = np.asarray(b_lin, np.float32)
    edge_emb = np.asarray(edge_emb, np.float32)
    W_heads = np.asarray(W_heads, np.float32)
    att_src = np.asarray(att_src, np.float32)
    att_dst = np.asarray(att_dst, np.float32)
    bias_heads = np.asarray(bias_heads, np.float32)

    order = np.argsort(dst, kind="stable")
    src_s = src[order].astype(np.int32)
    dst_s = dst[order].astype(np.int32)
    ew0_s = edge_emb[eids[order], 0]
    ddi_s = ddi[order]

    bounds = np.searchsorted(dst_s, np.arange(0, NPAD + NPW, NPW))
    K = 1
    for c in range(NCORES):
        for w in range(WPC):
            wi = c * WPC + w
            K = max(K, (int(bounds[wi + 1] - bounds[wi]) + 127) // 128)

    per_core = []
    for c in range(NCORES):
        srca = np.full((WPC * 128, K), PADIDX, np.int32)
        dstga = np.zeros((WPC * 128, K), np.int32)
        dstca = np.zeros((WPC * 128, K), np.float32)
        ewa = np.zeros((WPC * 128, K), np.float32)
        ddia = np.zeros((WPC * 128, K), np.float32)
        for w in range(WPC):
            wi = c * WPC + w
            e0, e1 = int(bounds[wi]), int(bounds[wi + 1])
            n = e1 - e0
            base = wi * NPW
            dstga[w * 128:(w + 1) * 128, :] = base
            if n == 0:
                continue
            j = np.arange(n)
            pp = w * 128 + (j % 128)
            tt = j // 128
            srca[pp, tt] = src_s[e0:e1]
            dstga[pp, tt] = dst_s[e0:e1]
            dstca[pp, tt] = (dst_s[e0:e1] - base).astype(np.float32)
            ewa[pp, tt] = ew0_s[e0:e1]
            ddia[pp, tt] = ddi_s[e0:e1]
        per_core.append(dict(srca=srca, dstg=dstga, dstc=dstca,
                             ewa=ewa, ddia=ddia))

    # weight folding (host): scores s = x @ (W_lin@asd) + b@asd
    asd = np.zeros((DIM, 6), np.float32)
    for h in range(N_HEADS):
        asd[:, h] = W_heads[h] @ att_src[h]
        asd[:, 3 + h] = W_heads[h] @ att_dst[h]
    wg = np.zeros((DIM, 138), np.float32)
    wg[:, 0:128] = W_lin
    wg[:, 128:134] = W_lin @ asd
    wg[:, 134:137] = wg[:, 131:134]          # duplicate s_dst cols for SDS
    bias_ext = np.zeros(138, np.float32)
    bias_ext[0:128] = b_lin
    bias_ext[128:134] = b_lin @ asd
    bias_ext[134:137] = bias_ext[131:134]
    biasr = np.tile(bias_ext, (128, 1)).astype(np.float32)

    xpad = np.zeros((NT1 * 128, DIM), np.float32)
    xpad[:N_NODES] = x
    # phase-1 matmul lhsT must be x^T per 128-node tile
    xpt = np.zeros((NT1 * 128, DIM), np.float32)
    for i in range(NT1):
        xpt[i * 128:(i + 1) * 128] = xpad[i * 128:(i + 1) * 128].T
    wheads2 = W_heads.reshape(N_HEADS * DIM, DIM).copy()
    bias3 = np.tile(bias_heads.sum(0) / N_HEADS, (128, 1)).astype(np.float32)
    iota = np.tile(np.arange(128, dtype=np.float32), (128, 1))
    poisonv = np.full((1, 3), -1e4, np.float32)

    shared = dict(xp=xpt, wg=wg, biasr=biasr, wheads=wheads2, bias3=bias3,
                  iota=iota, poison=poisonv)
    in_maps = []
    for c in range(NCORES):
        m = dict(shared)
        m.update(per_core[c])
        in_maps.append(m)
    return K, in_maps


def kernel(**inputs):
    K, in_maps = _prep(**inputs)
    if K not in _cache:
        _cache[K] = _build(K)
    nc = _cache[K]
    res = bass_utils.run_bass_kernel_spmd(nc, in_maps,
                                          core_ids=list(range(NCORES)))
    out = np.concatenate([res.results[c]["outc"] for c in range(NCORES)],
                         axis=0)
    return np.ascontiguousarray(out[:N_NODES]).astype(np.float32)
